# revision 3
# baseline (speedup 1.0000x reference)
"""Edge-parallel ExtractorMLP (gather + 3-layer MLP) for 8 TRN2 NeuronCores.

Strategy (pure edge parallelism, no cross-core communication):
  - 800K edges are split 100K per core; the embedding table and MLP
    weights are replicated.
  - The embedding table stays in HBM as [50000, 128] fp16 (256B per
    node row). Each 512-edge tile's two endpoint gathers run as SWDGE
    dma_gather instructions in transpose mode: the Q7 cores generate one
    256B descriptor per edge and the 16 DMA engines write the rows
    transposed into SBUF as [128 hidden, 512 edges] - exactly the matmul
    moving-operand layout. Descriptor generation sustains ~2.6ns/idx
    when gathers rotate across 4 SWDGE queues with ~12 outstanding
    buffers per side, so the gather stream (~540us/core) overlaps the
    tensor-engine MLP (~550us/core at full clock).
  - dma_gather indices are signed int16, so the node axis is split at
    25000 and each core's edges are bucketed by their (col_half,
    row_half) pair (4 buckets, <=3% padding). Each tile reads one fixed
    table half per endpoint with chunk-local int16 indices wrapped in
    16-partition groups. Edge order is restored on the host afterwards.
  - The MLP runs per 512-edge tile on the tensor engine in fp16 with
    fp32 PSUM accumulation: layer 1 as 4 M-chunks x 2 K-chunks, layer 2
    as 4 K-chunks, layer 3 as a single [128,1] stationary matmul.
    Bias+ReLU epilogues are split between the scalar (ACT) and vector
    (DVE) engines.
"""

from contextlib import ExitStack

import numpy as np

import concourse.bacc as bacc
import concourse.tile as tile
from concourse import mybir
from concourse.bass_utils import run_bass_kernel_spmd

P = 128
N = 512            # edges per tile (one fp32 PSUM bank)
IDXW = N // 16     # wrapped-index columns per tile
N_CORES = 8
N_NODES = 50000
N_EDGES = 800000
E_CORE = N_EDGES // N_CORES
HALF = 25000       # node-axis split so local indices fit signed int16
NQ = 4             # SWDGE queues to rotate gathers across

F16 = mybir.dt.float16
F32 = mybir.dt.float32
I16 = mybir.dt.int16


def _build_kernel(tile_map: tuple):
    """tile_map: per-tile (col_half, row_half) ids, compile-time."""
    nc = bacc.Bacc("TRN2", target_bir_lowering=False, debug=False,
                   num_swdge_queues=NQ)
    n_tiles = len(tile_map)

    tbl = nc.dram_tensor("tbl", [N_NODES, P], F16, kind="ExternalInput")
    colw = nc.dram_tensor("colw", [P, n_tiles * IDXW], I16, kind="ExternalInput")
    roww = nc.dram_tensor("roww", [P, n_tiles * IDXW], I16, kind="ExternalInput")
    w1 = nc.dram_tensor("w1", [P, 1024], F16, kind="ExternalInput")
    w2 = nc.dram_tensor("w2", [P, 512], F16, kind="ExternalInput")
    w3 = nc.dram_tensor("w3", [P, 1], F16, kind="ExternalInput")
    b1 = nc.dram_tensor("b1", [P, 4], F32, kind="ExternalInput")
    b2 = nc.dram_tensor("b2", [P, 1], F32, kind="ExternalInput")
    b3 = nc.dram_tensor("b3", [1, 1], F32, kind="ExternalInput")
    out = nc.dram_tensor("out", [n_tiles, N], F32, kind="ExternalOutput")

    Relu = mybir.ActivationFunctionType.Relu
    Identity = mybir.ActivationFunctionType.Identity

    with tile.TileContext(nc) as tc, ExitStack() as ctx:
        idxp = ctx.enter_context(tc.tile_pool(name="idxp", bufs=1))
        wp = ctx.enter_context(tc.tile_pool(name="wp", bufs=1))
        gcp = ctx.enter_context(tc.tile_pool(name="gcp", bufs=20))
        grp = ctx.enter_context(tc.tile_pool(name="grp", bufs=20))
        x1p = ctx.enter_context(tc.tile_pool(name="x1p", bufs=12))
        x2p = ctx.enter_context(tc.tile_pool(name="x2p", bufs=4))
        op = ctx.enter_context(tc.tile_pool(name="op", bufs=8))
        pl1 = ctx.enter_context(tc.tile_pool(name="pl1", bufs=5, space="PSUM"))
        pl2 = ctx.enter_context(tc.tile_pool(name="pl2", bufs=2, space="PSUM"))
        pl3 = ctx.enter_context(tc.tile_pool(name="pl3", bufs=1, space="PSUM"))

        # ---- one-time loads -------------------------------------------
        colw_sb = idxp.tile([P, n_tiles * IDXW], I16)
        roww_sb = idxp.tile([P, n_tiles * IDXW], I16)
        nc.scalar.dma_start(colw_sb[:], colw[:])
        nc.scalar.dma_start(roww_sb[:], roww[:])

        w1_sb = wp.tile([P, 1024], F16)
        w2_sb = wp.tile([P, 512], F16)
        w3_sb = wp.tile([P, 1], F16)
        b1_sb = wp.tile([P, 4], F32)
        b2_sb = wp.tile([P, 1], F32)
        b3_sb = wp.tile([1, 1], F32)
        nc.scalar.dma_start(w1_sb[:], w1[:])
        nc.scalar.dma_start(w2_sb[:], w2[:])
        nc.scalar.dma_start(w3_sb[:], w3[:])
        nc.scalar.dma_start(b1_sb[:], b1[:])
        nc.scalar.dma_start(b2_sb[:], b2[:])
        nc.scalar.dma_start(b3_sb[:], b3[:])

        # ---- steady state ---------------------------------------------
        for t in range(n_tiles):
            hc, hr = tile_map[t]
            isl = slice(t * IDXW, (t + 1) * IDXW)
            act_first = (t % 2 == 0)

            g_col = gcp.tile([P, N], F16, tag="gcol")
            nc.gpsimd.dma_gather(
                g_col[:].unsqueeze(1), tbl[hc * HALF:(hc + 1) * HALF, :],
                colw_sb[:, isl], N, N, P, transpose=True,
                queue_num=(2 * t) % NQ,
            )
            g_row = grp.tile([P, N], F16, tag="grow")
            nc.gpsimd.dma_gather(
                g_row[:].unsqueeze(1), tbl[hr * HALF:(hr + 1) * HALF, :],
                roww_sb[:, isl], N, N, P, transpose=True,
                queue_num=(2 * t + 1) % NQ,
            )

            # layer 1: [E,256] @ [256,512]; K-chunk 0 = col, 1 = row
            x1s = []
            for m in range(4):
                p1 = pl1.tile([P, N], F32, tag="pl1")
                nc.tensor.matmul(
                    p1[:], lhsT=w1_sb[:, m * 128:(m + 1) * 128],
                    rhs=g_col[:], start=True, stop=False,
                )
                nc.tensor.matmul(
                    p1[:], lhsT=w1_sb[:, 512 + m * 128: 512 + (m + 1) * 128],
                    rhs=g_row[:], start=False, stop=True,
                )
                x1 = x1p.tile([P, N], F16, tag="x1")
                if (m < 2) == act_first:
                    nc.scalar.activation(
                        x1[:], p1[:], Relu, bias=b1_sb[:, m:m + 1]
                    )
                else:
                    nc.vector.tensor_scalar(
                        out=x1[:], in0=p1[:],
                        scalar1=b1_sb[:, m:m + 1], scalar2=0.0,
                        op0=mybir.AluOpType.add, op1=mybir.AluOpType.max,
                    )
                x1s.append(x1)

            # layer 2: [E,512] @ [512,128]
            p2 = pl2.tile([P, N], F32, tag="pl2")
            for k in range(4):
                nc.tensor.matmul(
                    p2[:], lhsT=w2_sb[:, k * 128:(k + 1) * 128],
                    rhs=x1s[k][:], start=(k == 0), stop=(k == 3),
                )
            x2 = x2p.tile([P, N], F16, tag="x2")
            if act_first:
                nc.scalar.activation(x2[:], p2[:], Relu, bias=b2_sb[:, 0:1])
            else:
                nc.vector.tensor_scalar(
                    out=x2[:], in0=p2[:],
                    scalar1=b2_sb[:, 0:1], scalar2=0.0,
                    op0=mybir.AluOpType.add, op1=mybir.AluOpType.max,
                )

            # layer 3: [E,128] @ [128,1]
            p3 = pl3.tile([P, N], F32, tag="pl3")
            nc.tensor.matmul(p3[:1, :], lhsT=w3_sb[:], rhs=x2[:],
                             start=True, stop=True)
            o = op.tile([1, N], F32, tag="o")
            if act_first:
                nc.vector.tensor_scalar(
                    out=o[:1, :], in0=p3[:1, :], scalar1=b3_sb[:1, 0:1],
                    scalar2=None, op0=mybir.AluOpType.add,
                )
            else:
                nc.scalar.activation(o[:1, :], p3[:1, :], Identity,
                                     bias=b3_sb[:1, 0:1])
            nc.sync.dma_start(out[t:t + 1, :], o[:])

    nc.compile()
    return nc


def _wrap_indices(idx: np.ndarray) -> np.ndarray:
    """[n_tiles*512] local ids -> [128, n_tiles*32] int16 wrapped layout.

    dma_gather unwraps the first 16 partitions as rearrange("p s -> (s p)"),
    so index j of tile t sits at [16g + j%16, t*32 + j//16], replicated over
    the 8 groups g.
    """
    n_tiles = idx.shape[0] // N
    w = idx.astype(np.int16).reshape(n_tiles, IDXW, 16).transpose(0, 2, 1)
    w = np.tile(w, (1, 8, 1))
    return np.ascontiguousarray(w.transpose(1, 0, 2).reshape(P, n_tiles * IDXW))


def _bucketize(edge_index):
    """Bucket each core's edges by (col_half, row_half).

    Returns (tile_map, per-core [col_local, row_local, slot_orig]) where
    slot_orig maps padded slot -> original edge id within the core (-1 pad).
    """
    nb = 4
    cores = []
    counts = np.zeros((N_CORES, nb), np.int64)
    for c in range(N_CORES):
        sl = slice(c * E_CORE, (c + 1) * E_CORE)
        col = np.asarray(edge_index[0, sl], dtype=np.int64)
        row = np.asarray(edge_index[1, sl], dtype=np.int64)
        key = (col // HALF) * 2 + (row // HALF)
        order = np.argsort(key, kind="stable")
        counts[c] = np.bincount(key, minlength=nb)
        cores.append((col, row, key, order))

    tiles_per_bucket = np.ceil(counts.max(axis=0) / N).astype(np.int64)
    tile_map = []
    bucket_tile_start = np.zeros(nb, np.int64)
    for k in range(nb):
        bucket_tile_start[k] = len(tile_map)
        tile_map.extend([(k // 2, k % 2)] * int(tiles_per_bucket[k]))
    n_tiles = len(tile_map)

    per_core = []
    for c in range(N_CORES):
        col, row, key, order = cores[c]
        col_l = np.zeros(n_tiles * N, np.int64)
        row_l = np.zeros(n_tiles * N, np.int64)
        slot_orig = np.full(n_tiles * N, -1, np.int64)
        pos = 0
        for k in range(nb):
            nk = int(counts[c, k])
            if nk == 0:
                continue
            eids = order[pos:pos + nk]
            pos += nk
            base = int(bucket_tile_start[k]) * N
            hc, hr = k // 2, k % 2
            col_l[base:base + nk] = col[eids] - hc * HALF
            row_l[base:base + nk] = row[eids] - hr * HALF
            slot_orig[base:base + nk] = eids
        per_core.append((col_l, row_l, slot_orig))
    return tuple(tile_map), per_core


def _prep_shared(emb, W1, b1, W2, b2, W3, b3):
    return {
        "tbl": np.ascontiguousarray(emb.astype(np.float16)),
        "w1": np.ascontiguousarray(
            np.concatenate([W1[:128, :], W1[128:, :]], axis=1)
        ).astype(np.float16),
        "w2": np.ascontiguousarray(
            np.concatenate([W2[k * 128:(k + 1) * 128, :] for k in range(4)],
                           axis=1)
        ).astype(np.float16),
        "w3": W3.astype(np.float16),
        "b1": np.ascontiguousarray(b1.reshape(4, 128).T).astype(np.float32),
        "b2": b2[:, None].astype(np.float32),
        "b3": b3[None, :].astype(np.float32),
    }


_NC_CACHE = {}


def _get_nc(tile_map):
    key = tile_map
    if key not in _NC_CACHE:
        _NC_CACHE[key] = _build_kernel(tile_map)
    return _NC_CACHE[key]


def run(inputs: dict, trace: bool = False):
    """Run the kernel on 8 cores; returns (out [800000,1] f32, results)."""
    emb = np.asarray(inputs["emb"], dtype=np.float32)
    edge_index = np.asarray(inputs["edge_index"])
    shared = _prep_shared(
        emb,
        *[np.asarray(inputs[k], dtype=np.float32)
          for k in ("W1", "b1", "W2", "b2", "W3", "b3")]
    )
    tile_map, per_core = _bucketize(edge_index)
    in_maps = [
        dict(shared, colw=_wrap_indices(col_l), roww=_wrap_indices(row_l))
        for (col_l, row_l, _) in per_core
    ]
    nc = _get_nc(tile_map)
    res = run_bass_kernel_spmd(nc, in_maps, list(range(N_CORES)), trace=trace)
    out = np.empty((N_EDGES,), np.float32)
    for c in range(N_CORES):
        flat = res.results[c]["out"].reshape(-1)
        slot_orig = per_core[c][2]
        valid = slot_orig >= 0
        core_out = np.empty((E_CORE,), np.float32)
        core_out[slot_orig[valid]] = flat[valid]
        out[c * E_CORE:(c + 1) * E_CORE] = core_out
    return out[:, None], res


def kernel(**inputs) -> np.ndarray:
    out, _ = run(inputs, trace=False)
    return out
